# revision 1
# baseline (speedup 1.0000x reference)
"""Trainium2 Bass kernel for nn_Adj_layer (pairwise-diff conv stack + BN +
softmax + top-k masking), data-parallel over the batch axis on 8 NeuronCores.

Self-contained: hardcodes all shapes. Needs the concourse toolchain on the
python path (stock location /opt/trn_rl_repo inside the TRN2 container).
"""

import os
import sys

for _p in ("/opt/trn_rl_repo", os.path.expanduser("~/.axon_site/_ro/trn_rl_repo")):
    if os.path.isdir(_p) and _p not in sys.path:
        sys.path.insert(0, _p)

import numpy as np

import concourse.bacc as bacc
import concourse.bass as bass
import concourse.mybir as mybir
import concourse.tile as tile
from concourse.bass_utils import run_bass_kernel_spmd

F32 = mybir.dt.float32
AF = mybir.ActivationFunctionType
ALU = mybir.AluOpType

N_CORES = 8
B, V, D, H = 8, 201, 256, 128
NPIX = V * V                # 40401 pixels per batch element
NTOT = B * NPIX             # BN statistics population
K = 100                     # top-k
EPS = 1e-5
SLOPE = 0.01
CHUNK = 2 * V               # 402 pixels = 2 rows of the VxV map
NCHUNK = (NPIX + CHUNK - 1) // CHUNK   # 101 (last chunk = 1 row)
CIN = [D, 2 * H, 2 * H, H]  # per-block input channels
COUT = [2 * H, 2 * H, H, H]


def _build_nc(trace_scopes=False):
    nc = bacc.Bacc("TRN2", target_bir_lowering=False, num_devices=N_CORES)

    # ---- external I/O (per-core) ----
    xt_d = nc.dram_tensor("xt", [128, 2 * V], F32, kind="ExternalInput")
    w_d = [
        nc.dram_tensor("w0t", [128, 512], F32, kind="ExternalInput"),
        nc.dram_tensor("w1t", [128, 512], F32, kind="ExternalInput"),
        nc.dram_tensor("w2t", [128, 256], F32, kind="ExternalInput"),
        nc.dram_tensor("w3t", [128, 128], F32, kind="ExternalInput"),
    ]
    w4_d = nc.dram_tensor("w4t", [128, 1], F32, kind="ExternalInput")
    # per-block packed params: [g | be | b] each cout/128 columns
    p_d = [
        nc.dram_tensor(f"p{k}", [128, 3 * (COUT[k] // 128)], F32, kind="ExternalInput")
        for k in range(4)
    ]
    out_d = nc.dram_tensor("outb", [V, V], F32, kind="ExternalOutput")

    from contextlib import ExitStack
    with tile.TileContext(nc) as tc, ExitStack() as stack:
        dram = stack.enter_context(tc.tile_pool(name="dram", bufs=1, space="DRAM"))
        resid = stack.enter_context(tc.tile_pool(name="resid", bufs=1))
        psum = stack.enter_context(tc.tile_pool(name="psum", bufs=6, space="PSUM"))
        psum1 = stack.enter_context(tc.tile_pool(name="psum1", bufs=2, space="PSUM"))

        # internal DRAM: activation bounce buffers + logits
        ybuf = [[dram.tile([128, NPIX], F32, tag=f"y{k}_{ot}", name=f"y{k}_{ot}") for ot in range(2)]
                for k in range(2)]
        logits_d = dram.tile([V, V], F32, tag="logits", name="logits")
        ar_in = [dram.tile([128, 2 * (COUT[k] // 128)], F32, tag=f"arin{k}", name=f"arin{k}")
                 for k in range(4)]
        ar_out = [dram.tile([128, 2 * (COUT[k] // 128)], F32, tag=f"arout{k}", name=f"arout{k}")
                  for k in range(4)]

        # resident SBUF
        xT = resid.tile([128, 2 * V], F32, tag="xT", name="xT")
        wsb = [resid.tile([128, w_d[k].shape[1]], F32, tag=f"w{k}", name=f"wsb{k}") for k in range(4)]
        w4sb = resid.tile([128, 1], F32, tag="w4", name="w4sb")
        psb = [resid.tile([128, p_d[k].shape[1]], F32, tag=f"p{k}", name=f"psb{k}") for k in range(4)]
        bigbuf = resid.tile([128, NPIX], F32, tag="bigbuf", name="bigbuf")   # y2 then y3
        sumc = [resid.tile([128, NCHUNK], F32, tag=f"sumc{ot}", name=f"sumc{ot}") for ot in range(2)]
        sumsqc = [resid.tile([128, NCHUNK], F32, tag=f"sumsqc{ot}", name=f"sumsqc{ot}") for ot in range(2)]
        # per-block BN affine params
        s_sb = [resid.tile([128, COUT[k] // 128], F32, tag=f"s{k}", name=f"s_sb{k}") for k in range(4)]
        t_sb = [resid.tile([128, COUT[k] // 128], F32, tag=f"t{k}", name=f"t_sb{k}") for k in range(4)]

        nc.sync.dma_start(xT[:], xt_d[:])
        for k in range(4):
            nc.sync.dma_start(wsb[k][:], w_d[k][:])
            nc.sync.dma_start(psb[k][:], p_d[k][:])
        nc.sync.dma_start(w4sb[:], w4_d[:])

        def chunk_pixels(ch):
            n0 = ch * CHUNK
            return n0, min(CHUNK, NPIX - n0)

        def stats_and_store(k, ch, ps_tiles, store_fn, scr_pool):
            """Copy conv output (psum) to its destination and accumulate
            per-channel sum / sum-of-squares partials for chunk ch."""
            _, npx = chunk_pixels(ch)
            nt = COUT[k] // 128
            for ot in range(nt):
                dst = store_fn(ot)
                nc.scalar.activation(dst, ps_tiles[ot][:, :npx], AF.Copy,
                                     accum_out=sumc[ot][:, ch:ch + 1])
                scr = scr_pool.tile([128, CHUNK], F32, tag="scr", name="scr")
                nc.vector.scalar_tensor_tensor(
                    scr[:, :npx], dst, 1.0, dst,
                    op0=ALU.mult, op1=ALU.mult,
                    accum_out=sumsqc[ot][:, ch:ch + 1])

        def finalize_stats(k, work):
            """Column-reduce chunk partials, AllReduce across cores, compute
            BN affine s (scale) and t (shift) for block k."""
            nt = COUT[k] // 128
            sred = work.tile([128, 2 * nt], F32, tag="sred")
            for ot in range(nt):
                nc.vector.tensor_reduce(sred[:, ot:ot + 1], sumc[ot][:, :NCHUNK],
                                        axis=mybir.AxisListType.X, op=ALU.add)
                nc.vector.tensor_reduce(sred[:, nt + ot:nt + ot + 1],
                                        sumsqc[ot][:, :NCHUNK],
                                        axis=mybir.AxisListType.X, op=ALU.add)
            nc.gpsimd.dma_start(ar_in[k][:], sred[:])
            if os.environ.get("ADJ_NO_COLLECTIVE"):
                nc.gpsimd.dma_start(ar_out[k][:], ar_in[k][:])
            else:
                nc.gpsimd.collective_compute(
                    "AllReduce", ALU.add, replica_groups=[list(range(N_CORES))],
                    ins=[ar_in[k][:].opt()],
                    outs=[ar_out[k][:].opt()])
            gst = work.tile([128, 2 * nt], F32, tag="gst")
            nc.gpsimd.dma_start(gst[:], ar_out[k][:])
            mean = work.tile([128, nt], F32, tag="bn_mean")
            ey2 = work.tile([128, nt], F32, tag="bn_ey2")
            var = work.tile([128, nt], F32, tag="bn_var")
            sd = work.tile([128, nt], F32, tag="bn_sd")
            rd = work.tile([128, nt], F32, tag="bn_rd")
            tmp = work.tile([128, nt], F32, tag="bn_tmp")
            inv_n = 1.0 / float(NTOT)
            nc.vector.tensor_scalar_mul(mean[:], gst[:, 0:nt], inv_n)
            nc.vector.tensor_scalar_mul(ey2[:], gst[:, nt:2 * nt], inv_n)
            nc.vector.tensor_tensor(var[:], mean[:], mean[:], op=ALU.mult)
            nc.vector.tensor_tensor(var[:], ey2[:], var[:], op=ALU.subtract)
            nc.vector.tensor_scalar_add(var[:], var[:], EPS)
            nc.scalar.activation(sd[:], var[:], AF.Sqrt)
            nc.vector.reciprocal(rd[:], sd[:])
            g_ap = psb[k][:, 0:nt]
            be_ap = psb[k][:, nt:2 * nt]
            b_ap = psb[k][:, 2 * nt:3 * nt]
            nc.vector.tensor_tensor(s_sb[k][:], g_ap, rd[:], op=ALU.mult)
            # t = be - mean * s   (conv bias cancels inside batch-norm)
            nc.vector.tensor_tensor(tmp[:], mean[:], s_sb[k][:], op=ALU.mult)
            nc.vector.tensor_tensor(t_sb[k][:], be_ap, tmp[:], op=ALU.subtract)

        with tc.tile_pool(name="work", bufs=2) as work:
            # ================= phase 0: T = |x_i - x_j| -> conv0 -> y0 =====
            if trace_scopes:
                sc = nc.enter_named_scope("phase0")
            for ch in range(NCHUNK):
                n0, npx = chunk_pixels(ch)
                rows = [2 * ch, 2 * ch + 1][: (npx + V - 1) // V]
                tt = work.tile([128, 2 * CHUNK], F32, tag="tt")
                for ct in range(2):
                    for si, i in enumerate(rows):
                        nc.vector.tensor_scalar_sub(
                            tt[:, ct * CHUNK + si * V: ct * CHUNK + (si + 1) * V],
                            xT[:, ct * V:(ct + 1) * V],
                            xT[:, ct * V + i: ct * V + i + 1])
                for ct in range(2):
                    seg = tt[:, ct * CHUNK: ct * CHUNK + npx]
                    nc.scalar.activation(seg, seg, AF.Abs)
                ps = [psum.tile([128, CHUNK], F32, tag="ps", name="ps") for _ in range(2)]
                for ot in range(2):
                    for ct in range(2):
                        nc.tensor.matmul(
                            ps[ot][:, :npx],
                            wsb[0][:, ct * 256 + ot * 128: ct * 256 + (ot + 1) * 128],
                            tt[:, ct * CHUNK: ct * CHUNK + npx],
                            start=(ct == 0), stop=(ct == 1))
                stage = work.tile([128, 2 * CHUNK], F32, tag="stage")
                stats_and_store(0, ch, ps,
                                lambda ot: stage[:, ot * CHUNK: ot * CHUNK + npx],
                                work)
                for ot in range(2):
                    nc.sync.dma_start(ybuf[0][ot][:, n0:n0 + npx],
                                      stage[:, ot * CHUNK: ot * CHUNK + npx])
            finalize_stats(0, work)
            if trace_scopes:
                nc.leave_named_scope(sc)

            # ================= phases 1..3: conv blocks ====================
            for k in (1, 2, 3):
                if trace_scopes:
                    sc = nc.enter_named_scope(f"phase{k}")
                nti, nto = CIN[k] // 128, COUT[k] // 128
                for ch in range(NCHUNK):
                    n0, npx = chunk_pixels(ch)
                    # source of y_{k-1}
                    if k in (1, 2):
                        ysrc = work.tile([128, 2 * CHUNK], F32, tag="yin")
                        for ct in range(nti):
                            nc.sync.dma_start(
                                ysrc[:, ct * CHUNK: ct * CHUNK + npx],
                                ybuf[k - 1][ct][:, n0:n0 + npx])
                        src_ap = lambda ct: ysrc[:, ct * CHUNK: ct * CHUNK + npx]
                    else:
                        src_ap = lambda ct: bigbuf[:, n0:n0 + npx]
                    u = work.tile([128, 2 * CHUNK], F32, tag="u")
                    z = work.tile([128, 2 * CHUNK], F32, tag="z")
                    for ct in range(nti):
                        ua = u[:, ct * CHUNK: ct * CHUNK + npx]
                        za = z[:, ct * CHUNK: ct * CHUNK + npx]
                        nc.scalar.activation(ua, src_ap(ct), AF.Identity,
                                             bias=t_sb[k - 1][:, ct:ct + 1],
                                             scale=s_sb[k - 1][:, ct:ct + 1])
                        nc.vector.scalar_tensor_tensor(za, ua, SLOPE, ua,
                                                       op0=ALU.mult, op1=ALU.max)
                    ps = [psum.tile([128, CHUNK], F32, tag="ps", name="ps") for _ in range(nto)]
                    wk = wsb[k]
                    wct = COUT[k]  # columns per ct block in packed weight
                    for ot in range(nto):
                        for ct in range(nti):
                            nc.tensor.matmul(
                                ps[ot][:, :npx],
                                wk[:, ct * wct + ot * 128: ct * wct + (ot + 1) * 128],
                                z[:, ct * CHUNK: ct * CHUNK + npx],
                                start=(ct == 0), stop=(ct == nti - 1))
                    if k == 1:
                        stage = work.tile([128, 2 * CHUNK], F32, tag="stage")
                        stats_and_store(k, ch, ps,
                                        lambda ot: stage[:, ot * CHUNK: ot * CHUNK + npx],
                                        work)
                        for ot in range(nto):
                            nc.sync.dma_start(ybuf[1][ot][:, n0:n0 + npx],
                                              stage[:, ot * CHUNK: ot * CHUNK + npx])
                    else:
                        stats_and_store(k, ch, ps,
                                        lambda ot: bigbuf[:, n0:n0 + npx], work)
                finalize_stats(k, work)
                if trace_scopes:
                    nc.leave_named_scope(sc)

            # ================= phase 4: y3 -> logits =======================
            if trace_scopes:
                sc = nc.enter_named_scope("phase4")
            lg_flat = logits_d[:].rearrange("a b -> (a b)")
            for ch in range(NCHUNK):
                n0, npx = chunk_pixels(ch)
                u = work.tile([128, CHUNK], F32, tag="u", name="u")
                z = work.tile([128, CHUNK], F32, tag="z", name="z")
                nc.scalar.activation(u[:, :npx], bigbuf[:, n0:n0 + npx], AF.Identity,
                                     bias=t_sb[3][:, 0:1], scale=s_sb[3][:, 0:1])
                nc.vector.scalar_tensor_tensor(z[:, :npx], u[:, :npx], SLOPE,
                                               u[:, :npx], op0=ALU.mult, op1=ALU.max)
                lp = psum1.tile([1, CHUNK], F32, tag="lp")
                nc.tensor.matmul(lp[0:1, :npx], w4sb[:, 0:1], z[:, :npx],
                                 start=True, stop=True)
                lst = work.tile([1, CHUNK], F32, tag="stage", name="lst")
                nc.scalar.activation(lst[0:1, :npx], lp[0:1, :npx], AF.Copy)
                nc.sync.dma_start(lg_flat[n0:n0 + npx], lst[0:1, :npx])
            if trace_scopes:
                nc.leave_named_scope(sc)

        # ================= phase 5: softmax + topk mask ====================
        if trace_scopes:
            sc = nc.enter_named_scope("phase5")
        NR = (K // 8) + 1  # 13 max8 rounds to reach rank 100
        with tc.tile_pool(name="smax", bufs=2) as smax:
            for rt, (r0, nr) in enumerate([(0, 128), (128, V - 128)]):
                lt = smax.tile([128, V], F32, tag="lt")
                nc.sync.dma_start(lt[:nr, :], logits_d[r0:r0 + nr, :])
                lc = smax.tile([128, V], F32, tag="lc")
                nc.vector.tensor_copy(lc[:nr, :], lt[:nr, :])
                mx = smax.tile([128, 8 * NR], F32, tag="mx")
                for r in range(NR):
                    nc.vector.max(mx[:nr, 8 * r: 8 * (r + 1)], lc[:nr, :])
                    if r < NR - 1:
                        nc.vector.match_replace(lc[:nr, :],
                                                mx[:nr, 8 * r: 8 * (r + 1)],
                                                lc[:nr, :], -1e30)
                nmx = smax.tile([128, 1], F32, tag="nmx")
                nc.vector.tensor_scalar_mul(nmx[:nr, :], mx[:nr, 0:1], -1.0)
                et = smax.tile([128, V], F32, tag="et")
                rsum = smax.tile([128, 1], F32, tag="rsum")
                nc.scalar.activation(et[:nr, :], lt[:nr, :], AF.Exp,
                                     bias=nmx[:nr, 0:1], scale=1.0,
                                     accum_out=rsum[:nr, 0:1])
                rec = smax.tile([128, 1], F32, tag="rec")
                nc.vector.reciprocal(rec[:nr, :], rsum[:nr, :])
                pt = smax.tile([128, V], F32, tag="pt")
                nc.vector.tensor_scalar_mul(pt[:nr, :], et[:nr, :], rec[:nr, 0:1])
                ot_ = smax.tile([128, V], F32, tag="ot")
                nc.vector.scalar_tensor_tensor(ot_[:nr, :], lt[:nr, :],
                                               mx[:nr, K - 1:K], pt[:nr, :],
                                               op0=ALU.is_ge, op1=ALU.mult)
                nc.sync.dma_start(out_d[r0:r0 + nr, :], ot_[:nr, :])
        if trace_scopes:
            nc.leave_named_scope(sc)

    nc.finalize()
    return nc


def _prep_inputs(inputs):
    """Host-side reshape/transpose of the full inputs into per-core maps."""
    x = np.ascontiguousarray(inputs["x"], dtype=np.float32)

    def ctile(w):  # [cout, cin] -> [128, cin/128 * cout] packed per cin-tile
        wT = np.ascontiguousarray(w.T, dtype=np.float32)       # [cin, cout]
        cin, cout = wT.shape
        return np.ascontiguousarray(
            wT.reshape(cin // 128, 128, cout).transpose(1, 0, 2).reshape(128, -1))

    shared = {
        "w0t": ctile(inputs["w0"]), "w1t": ctile(inputs["w1"]),
        "w2t": ctile(inputs["w2"]), "w3t": ctile(inputs["w3"]),
        "w4t": ctile(inputs["w4"]),
    }

    def pcols(v):  # [cout] -> [128, cout/128]
        return np.ascontiguousarray(
            np.asarray(v, np.float32).reshape(-1, 128).T)

    for k in range(4):
        shared[f"p{k}"] = np.ascontiguousarray(np.concatenate(
            [pcols(inputs[f"g{k}"]), pcols(inputs[f"be{k}"]),
             pcols(inputs[f"b{k}"])], axis=1))

    in_maps = []
    for c in range(N_CORES):
        xt = np.ascontiguousarray(
            x[c].T.reshape(2, 128, V).transpose(1, 0, 2).reshape(128, 2 * V))
        in_maps.append({"xt": xt, **shared})
    return in_maps


_NC = None


def _get_nc():
    global _NC
    if _NC is None:
        _NC = _build_nc()
    return _NC


def kernel(**inputs):
    nc = _get_nc()
    in_maps = _prep_inputs(inputs)
    res = run_bass_kernel_spmd(nc, in_maps, core_ids=list(range(N_CORES)))
    return np.stack([res.results[c]["outb"] for c in range(N_CORES)], axis=0)



# revision 3
# speedup vs baseline: 302.0006x; 302.0006x over previous
"""Trainium2 Bass kernel for nn_Adj_layer (pairwise-diff conv stack + BN +
softmax + top-k masking), data-parallel over the batch axis on 8 NeuronCores.

Self-contained: hardcodes all shapes. Needs the concourse toolchain on the
python path (stock location /opt/trn_rl_repo inside the TRN2 container).
"""

import os
import sys

for _p in ("/opt/trn_rl_repo", os.path.expanduser("~/.axon_site/_ro/trn_rl_repo")):
    if os.path.isdir(_p) and _p not in sys.path:
        sys.path.insert(0, _p)

import numpy as np

import concourse.bacc as bacc
import concourse.bass as bass
import concourse.mybir as mybir
import concourse.tile as tile

F32 = mybir.dt.float32
AF = mybir.ActivationFunctionType
ALU = mybir.AluOpType

N_CORES = 8
B, V, D, H = 8, 201, 256, 128
NPIX = V * V                # 40401 pixels per batch element
NTOT = B * NPIX             # BN statistics population
K = 100                     # top-k
EPS = 1e-5
SLOPE = 0.01
CHUNK = 2 * V               # 402 pixels = 2 rows of the VxV map
NCHUNK = (NPIX + CHUNK - 1) // CHUNK   # 101 (last chunk = 1 row)
CIN = [D, 2 * H, 2 * H, H]  # per-block input channels
COUT = [2 * H, 2 * H, H, H]


def _build_nc(trace_scopes=False):
    nc = bacc.Bacc("TRN2", target_bir_lowering=False, num_devices=N_CORES)

    # ---- external I/O (per-core) ----
    xt_d = nc.dram_tensor("xt", [128, 2 * V], F32, kind="ExternalInput")
    w_d = [
        nc.dram_tensor("w0t", [128, 512], F32, kind="ExternalInput"),
        nc.dram_tensor("w1t", [128, 512], F32, kind="ExternalInput"),
        nc.dram_tensor("w2t", [128, 256], F32, kind="ExternalInput"),
        nc.dram_tensor("w3t", [128, 128], F32, kind="ExternalInput"),
    ]
    w4_d = nc.dram_tensor("w4t", [128, 1], F32, kind="ExternalInput")
    # per-block packed params: [g | be | b] each cout/128 columns
    p_d = [
        nc.dram_tensor(f"p{k}", [128, 3 * (COUT[k] // 128)], F32, kind="ExternalInput")
        for k in range(4)
    ]
    out_d = nc.dram_tensor("outb", [V, V], F32, kind="ExternalOutput")

    from contextlib import ExitStack
    with tile.TileContext(nc) as tc, ExitStack() as stack:
        dram = stack.enter_context(tc.tile_pool(name="dram", bufs=1, space="DRAM"))
        resid = stack.enter_context(tc.tile_pool(name="resid", bufs=1))
        psum = stack.enter_context(tc.tile_pool(name="psum", bufs=6, space="PSUM"))
        psum1 = stack.enter_context(tc.tile_pool(name="psum1", bufs=2, space="PSUM"))

        # internal DRAM: activation bounce buffers + logits
        ybuf = [[dram.tile([128, NPIX], F32, tag=f"y{k}_{ot}", name=f"y{k}_{ot}") for ot in range(2)]
                for k in range(2)]
        logits_d = dram.tile([V, V], F32, tag="logits", name="logits")
        ar_in = [dram.tile([128, 2 * (COUT[k] // 128)], F32, tag=f"arin{k}", name=f"arin{k}")
                 for k in range(4)]
        ar_out = [dram.tile([128, 2 * (COUT[k] // 128)], F32, tag=f"arout{k}", name=f"arout{k}")
                  for k in range(4)]

        # resident SBUF
        xT = resid.tile([128, 2 * V], F32, tag="xT", name="xT")
        wsb = [resid.tile([128, w_d[k].shape[1]], F32, tag=f"w{k}", name=f"wsb{k}") for k in range(4)]
        w4sb = resid.tile([128, 1], F32, tag="w4", name="w4sb")
        psb = [resid.tile([128, p_d[k].shape[1]], F32, tag=f"p{k}", name=f"psb{k}") for k in range(4)]
        bigbuf = resid.tile([128, NPIX], F32, tag="bigbuf", name="bigbuf")   # y2 then y3
        sumc = [resid.tile([128, NCHUNK], F32, tag=f"sumc{ot}", name=f"sumc{ot}") for ot in range(2)]
        sumsqc = [resid.tile([128, NCHUNK], F32, tag=f"sumsqc{ot}", name=f"sumsqc{ot}") for ot in range(2)]
        # per-block BN affine params
        s_sb = [resid.tile([128, COUT[k] // 128], F32, tag=f"s{k}", name=f"s_sb{k}") for k in range(4)]
        t_sb = [resid.tile([128, COUT[k] // 128], F32, tag=f"t{k}", name=f"t_sb{k}") for k in range(4)]

        nc.sync.dma_start(xT[:], xt_d[:])
        for k in range(4):
            nc.sync.dma_start(wsb[k][:], w_d[k][:])
            nc.sync.dma_start(psb[k][:], p_d[k][:])
        nc.sync.dma_start(w4sb[:], w4_d[:])

        def chunk_pixels(ch):
            n0 = ch * CHUNK
            return n0, min(CHUNK, NPIX - n0)

        def stats_and_store(k, ch, ps_tiles, store_fn, scr_pool):
            """Copy conv output (psum) to its destination and accumulate
            per-channel sum / sum-of-squares partials for chunk ch."""
            _, npx = chunk_pixels(ch)
            nt = COUT[k] // 128
            for ot in range(nt):
                dst = store_fn(ot)
                nc.scalar.activation(dst, ps_tiles[ot][:, :npx], AF.Copy,
                                     accum_out=sumc[ot][:, ch:ch + 1])
                scr = scr_pool.tile([128, CHUNK], F32, tag="scr", name="scr")
                nc.vector.scalar_tensor_tensor(
                    scr[:, :npx], dst, 1.0, dst,
                    op0=ALU.mult, op1=ALU.mult,
                    accum_out=sumsqc[ot][:, ch:ch + 1])

        def finalize_stats(k, work):
            """Column-reduce chunk partials, AllReduce across cores, compute
            BN affine s (scale) and t (shift) for block k."""
            nt = COUT[k] // 128
            sred = work.tile([128, 2 * nt], F32, tag="sred")
            for ot in range(nt):
                nc.vector.tensor_reduce(sred[:, ot:ot + 1], sumc[ot][:, :NCHUNK],
                                        axis=mybir.AxisListType.X, op=ALU.add)
                nc.vector.tensor_reduce(sred[:, nt + ot:nt + ot + 1],
                                        sumsqc[ot][:, :NCHUNK],
                                        axis=mybir.AxisListType.X, op=ALU.add)
            nc.gpsimd.dma_start(ar_in[k][:], sred[:])
            if os.environ.get("ADJ_NO_COLLECTIVE"):
                nc.gpsimd.dma_start(ar_out[k][:], ar_in[k][:])
            else:
                nc.gpsimd.collective_compute(
                    "AllReduce", ALU.add, replica_groups=[list(range(N_CORES))],
                    ins=[ar_in[k][:].opt()],
                    outs=[ar_out[k][:].opt()])
            gst = work.tile([128, 2 * nt], F32, tag="gst")
            nc.gpsimd.dma_start(gst[:], ar_out[k][:])
            mean = work.tile([128, nt], F32, tag="bn_mean")
            ey2 = work.tile([128, nt], F32, tag="bn_ey2")
            var = work.tile([128, nt], F32, tag="bn_var")
            sd = work.tile([128, nt], F32, tag="bn_sd")
            rd = work.tile([128, nt], F32, tag="bn_rd")
            tmp = work.tile([128, nt], F32, tag="bn_tmp")
            inv_n = 1.0 / float(NTOT)
            nc.vector.tensor_scalar_mul(mean[:], gst[:, 0:nt], inv_n)
            nc.vector.tensor_scalar_mul(ey2[:], gst[:, nt:2 * nt], inv_n)
            nc.vector.tensor_tensor(var[:], mean[:], mean[:], op=ALU.mult)
            nc.vector.tensor_tensor(var[:], ey2[:], var[:], op=ALU.subtract)
            nc.vector.tensor_scalar_add(var[:], var[:], EPS)
            nc.scalar.activation(sd[:], var[:], AF.Sqrt)
            nc.vector.reciprocal(rd[:], sd[:])
            g_ap = psb[k][:, 0:nt]
            be_ap = psb[k][:, nt:2 * nt]
            b_ap = psb[k][:, 2 * nt:3 * nt]
            nc.vector.tensor_tensor(s_sb[k][:], g_ap, rd[:], op=ALU.mult)
            # t = be - mean * s   (conv bias cancels inside batch-norm)
            nc.vector.tensor_tensor(tmp[:], mean[:], s_sb[k][:], op=ALU.mult)
            nc.vector.tensor_tensor(t_sb[k][:], be_ap, tmp[:], op=ALU.subtract)

        with tc.tile_pool(name="work", bufs=2) as work:
            # ================= phase 0: T = |x_i - x_j| -> conv0 -> y0 =====
            if trace_scopes:
                sc = nc.enter_named_scope("phase0")
            for ch in range(NCHUNK):
                n0, npx = chunk_pixels(ch)
                rows = [2 * ch, 2 * ch + 1][: (npx + V - 1) // V]
                tt = work.tile([128, 2 * CHUNK], F32, tag="tt")
                for ct in range(2):
                    for si, i in enumerate(rows):
                        nc.vector.tensor_scalar_sub(
                            tt[:, ct * CHUNK + si * V: ct * CHUNK + (si + 1) * V],
                            xT[:, ct * V:(ct + 1) * V],
                            xT[:, ct * V + i: ct * V + i + 1])
                for ct in range(2):
                    seg = tt[:, ct * CHUNK: ct * CHUNK + npx]
                    nc.scalar.activation(seg, seg, AF.Abs)
                ps = [psum.tile([128, CHUNK], F32, tag="ps", name="ps") for _ in range(2)]
                for ot in range(2):
                    for ct in range(2):
                        nc.tensor.matmul(
                            ps[ot][:, :npx],
                            wsb[0][:, ct * 256 + ot * 128: ct * 256 + (ot + 1) * 128],
                            tt[:, ct * CHUNK: ct * CHUNK + npx],
                            start=(ct == 0), stop=(ct == 1))
                stage = work.tile([128, 2 * CHUNK], F32, tag="stage")
                stats_and_store(0, ch, ps,
                                lambda ot: stage[:, ot * CHUNK: ot * CHUNK + npx],
                                work)
                for ot in range(2):
                    nc.sync.dma_start(ybuf[0][ot][:, n0:n0 + npx],
                                      stage[:, ot * CHUNK: ot * CHUNK + npx])
            finalize_stats(0, work)
            if trace_scopes:
                nc.leave_named_scope(sc)

            # ================= phases 1..3: conv blocks ====================
            for k in (1, 2, 3):
                if trace_scopes:
                    sc = nc.enter_named_scope(f"phase{k}")
                nti, nto = CIN[k] // 128, COUT[k] // 128
                for ch in range(NCHUNK):
                    n0, npx = chunk_pixels(ch)
                    # source of y_{k-1}
                    if k in (1, 2):
                        ysrc = work.tile([128, 2 * CHUNK], F32, tag="yin")
                        for ct in range(nti):
                            nc.sync.dma_start(
                                ysrc[:, ct * CHUNK: ct * CHUNK + npx],
                                ybuf[k - 1][ct][:, n0:n0 + npx])
                        src_ap = lambda ct: ysrc[:, ct * CHUNK: ct * CHUNK + npx]
                    else:
                        src_ap = lambda ct: bigbuf[:, n0:n0 + npx]
                    u = work.tile([128, 2 * CHUNK], F32, tag="u")
                    z = work.tile([128, 2 * CHUNK], F32, tag="z")
                    for ct in range(nti):
                        ua = u[:, ct * CHUNK: ct * CHUNK + npx]
                        za = z[:, ct * CHUNK: ct * CHUNK + npx]
                        nc.scalar.activation(ua, src_ap(ct), AF.Identity,
                                             bias=t_sb[k - 1][:, ct:ct + 1],
                                             scale=s_sb[k - 1][:, ct:ct + 1])
                        nc.vector.scalar_tensor_tensor(za, ua, SLOPE, ua,
                                                       op0=ALU.mult, op1=ALU.max)
                    ps = [psum.tile([128, CHUNK], F32, tag="ps", name="ps") for _ in range(nto)]
                    wk = wsb[k]
                    wct = COUT[k]  # columns per ct block in packed weight
                    for ot in range(nto):
                        for ct in range(nti):
                            nc.tensor.matmul(
                                ps[ot][:, :npx],
                                wk[:, ct * wct + ot * 128: ct * wct + (ot + 1) * 128],
                                z[:, ct * CHUNK: ct * CHUNK + npx],
                                start=(ct == 0), stop=(ct == nti - 1))
                    if k == 1:
                        stage = work.tile([128, 2 * CHUNK], F32, tag="stage")
                        stats_and_store(k, ch, ps,
                                        lambda ot: stage[:, ot * CHUNK: ot * CHUNK + npx],
                                        work)
                        for ot in range(nto):
                            nc.sync.dma_start(ybuf[1][ot][:, n0:n0 + npx],
                                              stage[:, ot * CHUNK: ot * CHUNK + npx])
                    else:
                        stats_and_store(k, ch, ps,
                                        lambda ot: bigbuf[:, n0:n0 + npx], work)
                finalize_stats(k, work)
                if trace_scopes:
                    nc.leave_named_scope(sc)

            # ================= phase 4: y3 -> logits =======================
            if trace_scopes:
                sc = nc.enter_named_scope("phase4")
            lg_flat = logits_d[:].rearrange("a b -> (a b)")
            for ch in range(NCHUNK):
                n0, npx = chunk_pixels(ch)
                u = work.tile([128, CHUNK], F32, tag="u", name="u")
                z = work.tile([128, CHUNK], F32, tag="z", name="z")
                nc.scalar.activation(u[:, :npx], bigbuf[:, n0:n0 + npx], AF.Identity,
                                     bias=t_sb[3][:, 0:1], scale=s_sb[3][:, 0:1])
                nc.vector.scalar_tensor_tensor(z[:, :npx], u[:, :npx], SLOPE,
                                               u[:, :npx], op0=ALU.mult, op1=ALU.max)
                lp = psum1.tile([1, CHUNK], F32, tag="lp")
                nc.tensor.matmul(lp[0:1, :npx], w4sb[:, 0:1], z[:, :npx],
                                 start=True, stop=True)
                lst = work.tile([1, CHUNK], F32, tag="stage", name="lst")
                nc.scalar.activation(lst[0:1, :npx], lp[0:1, :npx], AF.Copy)
                nc.sync.dma_start(lg_flat[n0:n0 + npx], lst[0:1, :npx])
            if trace_scopes:
                nc.leave_named_scope(sc)

        # ================= phase 5: softmax + topk mask ====================
        if trace_scopes:
            sc = nc.enter_named_scope("phase5")
        NR = (K // 8) + 1  # 13 max8 rounds to reach rank 100
        with tc.tile_pool(name="smax", bufs=2) as smax:
            for rt, (r0, nr) in enumerate([(0, 128), (128, V - 128)]):
                lt = smax.tile([128, V], F32, tag="lt")
                nc.sync.dma_start(lt[:nr, :], logits_d[r0:r0 + nr, :])
                lc = smax.tile([128, V], F32, tag="lc")
                nc.vector.tensor_copy(lc[:nr, :], lt[:nr, :])
                mx = smax.tile([128, 8 * NR], F32, tag="mx")
                for r in range(NR):
                    nc.vector.max(mx[:nr, 8 * r: 8 * (r + 1)], lc[:nr, :])
                    if r < NR - 1:
                        nc.vector.match_replace(lc[:nr, :],
                                                mx[:nr, 8 * r: 8 * (r + 1)],
                                                lc[:nr, :], -1e30)
                nmx = smax.tile([128, 1], F32, tag="nmx")
                nc.vector.tensor_scalar_mul(nmx[:nr, :], mx[:nr, 0:1], -1.0)
                et = smax.tile([128, V], F32, tag="et")
                rsum = smax.tile([128, 1], F32, tag="rsum")
                nc.scalar.activation(et[:nr, :], lt[:nr, :], AF.Exp,
                                     bias=nmx[:nr, 0:1], scale=1.0,
                                     accum_out=rsum[:nr, 0:1])
                rec = smax.tile([128, 1], F32, tag="rec")
                nc.vector.reciprocal(rec[:nr, :], rsum[:nr, :])
                pt = smax.tile([128, V], F32, tag="pt")
                nc.vector.tensor_scalar_mul(pt[:nr, :], et[:nr, :], rec[:nr, 0:1])
                ot_ = smax.tile([128, V], F32, tag="ot")
                nc.vector.scalar_tensor_tensor(ot_[:nr, :], lt[:nr, :],
                                               mx[:nr, K - 1:K], pt[:nr, :],
                                               op0=ALU.is_ge, op1=ALU.mult)
                nc.sync.dma_start(out_d[r0:r0 + nr, :], ot_[:nr, :])
        if trace_scopes:
            nc.leave_named_scope(sc)

    nc.finalize()
    return nc


def _prep_inputs(inputs):
    """Host-side reshape/transpose of the full inputs into per-core maps."""
    x = np.ascontiguousarray(inputs["x"], dtype=np.float32)

    def ctile(w):  # [cout, cin] -> [128, cin/128 * cout] packed per cin-tile
        wT = np.ascontiguousarray(w.T, dtype=np.float32)       # [cin, cout]
        cin, cout = wT.shape
        return np.ascontiguousarray(
            wT.reshape(cin // 128, 128, cout).transpose(1, 0, 2).reshape(128, -1))

    shared = {
        "w0t": ctile(inputs["w0"]), "w1t": ctile(inputs["w1"]),
        "w2t": ctile(inputs["w2"]), "w3t": ctile(inputs["w3"]),
        "w4t": ctile(inputs["w4"]),
    }

    def pcols(v):  # [cout] -> [128, cout/128]
        return np.ascontiguousarray(
            np.asarray(v, np.float32).reshape(-1, 128).T)

    for k in range(4):
        shared[f"p{k}"] = np.ascontiguousarray(np.concatenate(
            [pcols(inputs[f"g{k}"]), pcols(inputs[f"be{k}"]),
             pcols(inputs[f"b{k}"])], axis=1))

    in_maps = []
    for c in range(N_CORES):
        xt = np.ascontiguousarray(
            x[c].T.reshape(2, 128, V).transpose(1, 0, 2).reshape(128, 2 * V))
        in_maps.append({"xt": xt, **shared})
    return in_maps


class _Runner:
    """Cached PJRT executor for the bass module.

    Functionally the same axon path as bass_utils.run_bass_kernel_spmd
    (shard_map over 8 neuron devices + bass_exec custom call), but the
    jitted executable is built ONCE and reused — run_bass_kernel_spmd
    rebuilds the jax.jit closure per call, paying a full retrace/relower
    (~700ms) on every invocation. The donated-zero-output trick is also
    dropped: this kernel writes every element of its output, so the
    dummy output-shaped operands can be persistent device arrays instead
    of per-call zero uploads.
    """

    def __init__(self, nc):
        import jax
        from jax.sharding import Mesh, PartitionSpec, NamedSharding
        from jax.experimental.shard_map import shard_map
        from concourse.bass2jax import (
            _bass_exec_p,
            partition_id_tensor,
            install_neuronx_cc_hook,
        )

        install_neuronx_cc_hook()
        self.jax = jax
        self.nc = nc
        if nc.dbg_addr is not None and nc.dbg_callbacks:
            raise RuntimeError("dbg callbacks unsupported under axon")
        self.dbg_name = nc.dbg_addr.name if nc.dbg_addr is not None else None

        partition_name = (
            nc.partition_id_tensor.name if nc.partition_id_tensor else None
        )
        in_names, out_names, out_avals, zero_shapes = [], [], [], []
        for alloc in nc.m.functions[0].allocations:
            if not isinstance(alloc, mybir.MemoryLocationSet):
                continue
            name = alloc.memorylocations[0].name
            if alloc.kind == "ExternalInput":
                if name != partition_name:
                    in_names.append(name)
            elif alloc.kind == "ExternalOutput":
                out_names.append(name)
                shape = tuple(alloc.tensor_shape)
                dtype = mybir.dt.np(alloc.dtype)
                out_avals.append(jax.core.ShapedArray(shape, dtype))
                zero_shapes.append((shape, dtype))
        n_params = len(in_names)
        n_outs = len(out_avals)
        all_in_names = list(in_names) + list(out_names)
        if partition_name is not None:
            all_in_names.append(partition_name)
        self.in_names, self.out_names, self.out_avals = in_names, out_names, out_avals

        devices = jax.devices()[:N_CORES]
        assert len(devices) == N_CORES
        self.mesh = Mesh(np.asarray(devices), ("core",))
        self.sharding = NamedSharding(self.mesh, PartitionSpec("core"))

        def _body(*args):
            operands = list(args)
            if partition_name is not None:
                operands.append(partition_id_tensor())
            return tuple(
                _bass_exec_p.bind(
                    *operands,
                    out_avals=tuple(out_avals),
                    in_names=tuple(all_in_names),
                    out_names=tuple(out_names),
                    lowering_input_output_aliases=(),
                    sim_require_finite=True,
                    sim_require_nnan=True,
                    nc=nc,
                )
            )

        self._sharded = jax.jit(
            shard_map(
                _body,
                mesh=self.mesh,
                in_specs=(PartitionSpec("core"),) * (n_params + n_outs),
                out_specs=(PartitionSpec("core"),) * n_outs,
                check_rep=False,
            ),
            keep_unused=True,
        )
        # persistent dummy operands for the output slots (never read: the
        # kernel fully writes its outputs)
        self._zeros = [
            jax.device_put(
                np.zeros((N_CORES * s[0],) + tuple(s[1:]), dt), self.sharding
            )
            for (s, dt) in zero_shapes
        ]

    def concat_inputs(self, in_maps):
        if self.dbg_name is not None:
            dbg = np.zeros((1, 2), np.uint32)
            in_maps = [{**m, self.dbg_name: dbg} for m in in_maps]
        return [
            np.concatenate([np.asarray(m[nm]) for m in in_maps], axis=0)
            for nm in self.in_names
        ]

    def put_inputs(self, concat_in):
        return [self.jax.device_put(a, self.sharding) for a in concat_in]

    def dispatch(self, dev_in):
        """one kernel execution on the 8 cores (async; returns device arrays)"""
        return self._sharded(*dev_in, *self._zeros)

    def run_full(self, in_maps):
        outs = self.dispatch(self.put_inputs(self.concat_inputs(in_maps)))
        n = N_CORES
        return [
            {
                nm: np.asarray(outs[i]).reshape(n, *self.out_avals[i].shape)[c]
                for i, nm in enumerate(self.out_names)
            }
            for c in range(n)
        ]


_RUNNER = None


def _get_runner():
    global _RUNNER
    if _RUNNER is None:
        _RUNNER = _Runner(_build_nc())
    return _RUNNER


def kernel(**inputs):
    r = _get_runner()
    res = r.run_full(_prep_inputs(inputs))
    return np.stack([res[c]["outb"] for c in range(N_CORES)], axis=0)



# revision 20
# speedup vs baseline: 544.0117x; 1.8014x over previous
"""Trainium2 Bass kernel for nn_Adj_layer (pairwise-diff conv stack + BN +
softmax + top-k masking), data-parallel over the batch axis on 8 NeuronCores.

Self-contained: hardcodes all shapes. Needs the concourse toolchain on the
python path (stock location /opt/trn_rl_repo inside the TRN2 container).
"""

import os
import sys

for _p in ("/opt/trn_rl_repo", os.path.expanduser("~/.axon_site/_ro/trn_rl_repo")):
    if os.path.isdir(_p) and _p not in sys.path:
        sys.path.insert(0, _p)

import numpy as np

import concourse.bacc as bacc
import concourse.bass as bass
import concourse.mybir as mybir
import concourse.tile as tile

F32 = mybir.dt.float32
BF16 = mybir.dt.bfloat16
AF = mybir.ActivationFunctionType
ALU = mybir.AluOpType

N_CORES = 8
B, V, D, H = 8, 201, 256, 128
NPIX = V * V                # 40401 pixels per batch element
NTOT = B * NPIX             # BN statistics population
K = 100                     # top-k
EPS = 1e-5
SLOPE = 0.01
CIN = [D, 2 * H, 2 * H, H]  # per-block input channels
COUT = [2 * H, 2 * H, H, H]

# T = |x_i - x_j| is symmetric in (i, j) and the conv stack is per-pixel, so
# only the upper triangle (j >= i) is computed; the logit matrix is mirrored
# before the row softmax. Row i (width V-i) pairs with row V-1-i (width i+1)
# for a constant 202-pixel chunk.
NTRI = V * (V + 1) // 2     # 20301 upper-tri pixels (incl diag)
CHUNK = V + 1               # 202
NCHUNK = 101                # chunks 0..99 are 202 px; chunk 100 is 101 px

# packed per-core input [128, NCOLS] (f32): x^T | conv weights | bn params
_XT0 = 0
_W0 = 2 * V                          # 402
_W1 = _W0 + 512                      # 914
_W2 = _W1 + 512                      # 1426
_W3 = _W2 + 256                      # 1682
_W4 = _W3 + 128                      # 1810
_P0 = _W4 + 1                        # 1811
_P1 = _P0 + 4                        # 1815
_P2 = _P1 + 4                        # 1819
_P3 = _P2 + 2                        # 1821
_IDC = _P3 + 2                       # 1823  128x128 identity (PE transpose)
_M0 = _IDC + 128                     # 1951  strict-lower mask rows 0..127
_M1 = _M0 + V                        # 2152  strict-lower mask rows 128..200
NCOLS = _M1 + V                      # 2353
_WOFF = [_W0, _W1, _W2, _W3]
_POFF = [_P0, _P1, _P2, _P3]


def _chunk_geom(i):
    """pixel offset, width, and row segments [(row, col0, width, pos)]"""
    if i < 100:
        wA = V - i
        return CHUNK * i, CHUNK, [(i, i, wA, 0), (200 - i, 200 - i, i + 1, wA)]
    return CHUNK * 100, 101, [(100, 100, 101, 0)]


def _build_nc(trace_scopes=False):
    nc = bacc.Bacc("TRN2", target_bir_lowering=False, num_devices=N_CORES)

    # ---- external I/O (per-core) ----
    pk_d = nc.dram_tensor("pk", [128, NCOLS], F32, kind="ExternalInput")
    out_d = nc.dram_tensor("outb", [V, V], F32, kind="ExternalOutput")

    from contextlib import ExitStack
    with tile.TileContext(nc) as tc, ExitStack() as stack:
        dram = stack.enter_context(tc.tile_pool(name="dram", bufs=1, space="DRAM"))
        resid = stack.enter_context(tc.tile_pool(name="resid", bufs=1))
        psum = stack.enter_context(tc.tile_pool(name="psum", bufs=6, space="PSUM"))
        psum1 = stack.enter_context(tc.tile_pool(name="psum1", bufs=2, space="PSUM"))

        # internal DRAM: bf16 activation bounce buffers (chunk-major: the two
        # channel tiles of chunk i live at [2*n0, 2*n0+2*npx)), the upper-tri
        # logit matrix, and the tiny AllReduce buffers
        y0d = dram.tile([128, 2 * NTRI], F32, tag="y0d", name="y0d")
        y1d = dram.tile([128, 2 * NTRI], F32, tag="y1d", name="y1d")
        y2d = dram.tile([128, NTRI], F32, tag="y2d", name="y2d")
        u_d = dram.tile([V, V], F32, tag="ud", name="ud")
        ar_in = [dram.tile([128, 2 * (COUT[k] // 128)], F32, tag=f"arin{k}", name=f"arin{k}")
                 for k in range(4)]
        ar_out = [dram.tile([128, 2 * (COUT[k] // 128)], F32, tag=f"arout{k}", name=f"arout{k}")
                  for k in range(4)]

        # resident SBUF
        pksb = resid.tile([128, NCOLS], F32, tag="pk", name="pksb")
        negx = resid.tile([128, 2 * V], F32, tag="negx", name="negx")
        y3sb = resid.tile([128, NTRI], F32, tag="y3sb", name="y3sb")
        sumc = [resid.tile([128, NCHUNK], F32, tag=f"sumc{ot}", name=f"sumc{ot}") for ot in range(2)]
        sumsqc = [resid.tile([128, NCHUNK], F32, tag=f"sumsqc{ot}", name=f"sumsqc{ot}") for ot in range(2)]
        s_sb = [resid.tile([128, COUT[k] // 128], F32, tag=f"s{k}", name=f"s_sb{k}") for k in range(4)]
        t_sb = [resid.tile([128, COUT[k] // 128], F32, tag=f"t{k}", name=f"t_sb{k}") for k in range(4)]
        ydiag = [resid.tile([128, COUT[k] // 128], F32, tag=f"yd{k}", name=f"ydiag{k}")
                 for k in range(4)]
        zrow = resid.tile([128, V], F32, tag="zrow", name="zrow")

        nc.sync.dma_start(pksb[:], pk_d[:])
        xT = pksb[:, _XT0:_XT0 + 2 * V]
        # fp32 conv weights are used straight out of the packed tile: the
        # bf16 variant flipped ~45% of the top-k boundary rows (rel err
        # 4.6e-2 > 2e-2), so matmuls stay fp32
        nc.vector.tensor_scalar_mul(negx[:], xT, -1.0)
        # identity / strict-lower masks come in with the packed input (the
        # Pool engine cannot run affine_select/iota on this toolchain)
        ident = pksb[:, _IDC:_IDC + 128]
        mask0 = pksb[:, _M0:_M0 + V]
        mask1 = pksb[:, _M1:_M1 + V]
        nc.gpsimd.memset(zrow[:], 0.0)
        # pre-zero U so its lower triangle reads back as exact zeros
        nc.sync.dma_start(u_d[0:128, :], zrow[:, 0:V])
        nc.sync.dma_start(u_d[128:V, :], zrow[0:V - 128, 0:V])

        def finalize_stats(k, work):
            """Column-reduce chunk partials, mirror the triangle stats to the
            full V x V population (full = 2*tri - 201*ydiag), AllReduce across
            cores, then compute BN affine s/t for block k."""
            nt = COUT[k] // 128
            sred = work.tile([128, 2 * nt], F32, tag="sred")
            for ot in range(nt):
                nc.vector.tensor_reduce(sred[:, ot:ot + 1], sumc[ot][:, :NCHUNK],
                                        axis=mybir.AxisListType.X, op=ALU.add)
                nc.vector.tensor_reduce(sred[:, nt + ot:nt + ot + 1],
                                        sumsqc[ot][:, :NCHUNK],
                                        axis=mybir.AxisListType.X, op=ALU.add)
            sredc = work.tile([128, 2 * nt], F32, tag="sredc")
            if k == 0:
                # diag pixels of block 0 are exactly zero: full = 2*tri
                nc.vector.tensor_scalar_mul(sredc[:], sred[:], 2.0)
            else:
                corr = work.tile([128, 2 * nt], F32, tag="corr")
                nc.vector.tensor_scalar_mul(corr[:, 0:nt], ydiag[k][:], float(V))
                nc.vector.scalar_tensor_tensor(corr[:, nt:2 * nt], ydiag[k][:],
                                               float(V), ydiag[k][:],
                                               op0=ALU.mult, op1=ALU.mult)
                nc.vector.scalar_tensor_tensor(sredc[:], sred[:], 2.0, corr[:],
                                               op0=ALU.mult, op1=ALU.subtract)
            nc.gpsimd.dma_start(ar_in[k][:], sredc[:])
            if os.environ.get("ADJ_NO_COLLECTIVE"):
                nc.gpsimd.dma_start(ar_out[k][:], ar_in[k][:])
            else:
                nc.gpsimd.collective_compute(
                    "AllReduce", ALU.add, replica_groups=[list(range(N_CORES))],
                    ins=[ar_in[k][:].opt()],
                    outs=[ar_out[k][:].opt()])
            gst = work.tile([128, 2 * nt], F32, tag="gst")
            nc.gpsimd.dma_start(gst[:], ar_out[k][:])
            mean = work.tile([128, nt], F32, tag="bn_mean")
            ey2 = work.tile([128, nt], F32, tag="bn_ey2")
            var = work.tile([128, nt], F32, tag="bn_var")
            sd = work.tile([128, nt], F32, tag="bn_sd")
            rd = work.tile([128, nt], F32, tag="bn_rd")
            tmp = work.tile([128, nt], F32, tag="bn_tmp")
            inv_n = 1.0 / float(NTOT)
            nc.vector.tensor_scalar_mul(mean[:], gst[:, 0:nt], inv_n)
            nc.vector.tensor_scalar_mul(ey2[:], gst[:, nt:2 * nt], inv_n)
            nc.vector.tensor_tensor(var[:], mean[:], mean[:], op=ALU.mult)
            nc.vector.tensor_tensor(var[:], ey2[:], var[:], op=ALU.subtract)
            nc.vector.tensor_scalar_add(var[:], var[:], EPS)
            nc.scalar.activation(sd[:], var[:], AF.Sqrt)
            nc.vector.reciprocal(rd[:], sd[:])
            g_ap = pksb[:, _POFF[k]:_POFF[k] + nt]
            be_ap = pksb[:, _POFF[k] + nt:_POFF[k] + 2 * nt]
            nc.vector.tensor_tensor(s_sb[k][:], g_ap, rd[:], op=ALU.mult)
            # t = be - mean * s   (conv bias cancels inside batch-norm)
            nc.vector.tensor_tensor(tmp[:], mean[:], s_sb[k][:], op=ALU.mult)
            nc.vector.tensor_tensor(t_sb[k][:], be_ap, tmp[:], op=ALU.subtract)

        def stats_and_store(k, ch, npx, ps_tiles, store_fn):
            """Move conv output (psum, f32) to its bf16 destination and
            accumulate per-channel sum / sum-of-squares partials. Store+sum
            run on the vector engine (psum is read once per instruction),
            squares on gpsimd from the stored tile, so the scalar engine
            keeps only the activations; capture the diagonal value on chunk 0
            for the triangle-to-full stats correction."""
            nt = COUT[k] // 128
            for ot in range(nt):
                dst = store_fn(ot)
                nc.vector.tensor_copy(dst, ps_tiles[ot][:, :npx])
                nc.vector.tensor_reduce(sumc[ot][:, ch:ch + 1],
                                        ps_tiles[ot][:, :npx],
                                        axis=mybir.AxisListType.X, op=ALU.add)
                if k < 2:
                    nc.vector.scalar_tensor_tensor(
                        scr_of(ot, npx), dst, 1.0, dst,
                        op0=ALU.mult, op1=ALU.mult,
                        accum_out=sumsqc[ot][:, ch:ch + 1])
                else:
                    nc.scalar.activation(
                        scr_of(ot, npx), ps_tiles[ot][:, :npx], AF.Square,
                        accum_out=sumsqc[ot][:, ch:ch + 1])
                if ch == 0 and k > 0:
                    nc.vector.tensor_copy(ydiag[k][:, ot:ot + 1],
                                          ps_tiles[ot][:, 0:1])

        with tc.tile_pool(name="work", bufs=3) as work:
            def scr_of(ot, npx):
                scr = work.tile([128, CHUNK], BF16, tag=f"scr{ot}")
                return scr[:, :npx]

            # ===== phase 0: T = |x_i - x_j| -> conv0 -> y0 (upper tri) =====
            if trace_scopes:
                sc = nc.enter_named_scope("phase0")
            for ch in range(NCHUNK):
                n0, npx, segs = _chunk_geom(ch)
                tt = work.tile([128, 2 * CHUNK], F32, tag="tt")
                for ct in range(2):
                    for (row, col0, w, pos) in segs:
                        nc.scalar.activation(
                            tt[:, ct * npx + pos: ct * npx + pos + w],
                            xT[:, ct * V + col0: (ct + 1) * V],
                            AF.Abs,
                            bias=negx[:, ct * V + row: ct * V + row + 1])
                ps = [psum.tile([128, CHUNK], F32, tag="ps", name="ps") for _ in range(2)]
                for ot in range(2):
                    for ct in range(2):
                        nc.tensor.matmul(
                            ps[ot][:, :npx],
                            pksb[:, _W0 + ct * 256 + ot * 128: _W0 + ct * 256 + (ot + 1) * 128],
                            tt[:, ct * npx: (ct + 1) * npx],
                            start=(ct == 0), stop=(ct == 1))
                stage = work.tile([128, 2 * CHUNK], F32, tag="stage")
                stats_and_store(0, ch, npx, ps,
                                lambda ot: stage[:, ot * npx: (ot + 1) * npx])
                nc.gpsimd.dma_start(y0d[:, 2 * n0: 2 * n0 + 2 * npx],
                                    stage[:, :2 * npx])
            finalize_stats(0, work)
            if trace_scopes:
                nc.leave_named_scope(sc)

            # ================= phases 1..3: conv blocks ====================
            for k in (1, 2, 3):
                if trace_scopes:
                    sc = nc.enter_named_scope(f"phase{k}")
                nti, nto = CIN[k] // 128, COUT[k] // 128
                ysrc_d = y0d if k == 1 else y1d
                for ch in range(NCHUNK):
                    n0, npx, _ = _chunk_geom(ch)
                    if k in (1, 2):
                        ysrc = work.tile([128, 2 * CHUNK], F32, tag="yin")
                        nc.sync.dma_start(ysrc[:, :2 * npx],
                                          ysrc_d[:, 2 * n0: 2 * n0 + 2 * npx])
                        src_ap = lambda ct: ysrc[:, ct * npx: (ct + 1) * npx]
                    else:
                        ysrc = work.tile([128, 2 * CHUNK], F32, tag="yin")
                        nc.sync.dma_start(ysrc[:, :npx], y2d[:, n0:n0 + npx])
                        src_ap = lambda ct: ysrc[:, :npx]
                    z = work.tile([128, 2 * CHUNK], F32, tag="z")
                    for ct in range(nti):
                        nc.scalar.activation(z[:, ct * npx: (ct + 1) * npx],
                                             src_ap(ct), AF.Lrelu,
                                             bias=t_sb[k - 1][:, ct:ct + 1],
                                             scale=s_sb[k - 1][:, ct:ct + 1],
                                             alpha=SLOPE)
                    ps = [psum.tile([128, CHUNK], F32, tag="ps", name="ps") for _ in range(nto)]
                    wct = COUT[k]  # columns per ct block in packed weight
                    for ot in range(nto):
                        for ct in range(nti):
                            nc.tensor.matmul(
                                ps[ot][:, :npx],
                                pksb[:, _WOFF[k] + ct * wct + ot * 128:
                                      _WOFF[k] + ct * wct + (ot + 1) * 128],
                                z[:, ct * npx: (ct + 1) * npx],
                                start=(ct == 0), stop=(ct == nti - 1))
                    if k == 1:
                        stage = work.tile([128, 2 * CHUNK], F32, tag="stage")
                        stats_and_store(k, ch, npx, ps,
                                        lambda ot: stage[:, ot * npx: (ot + 1) * npx])
                        nc.gpsimd.dma_start(y1d[:, 2 * n0: 2 * n0 + 2 * npx],
                                            stage[:, :2 * npx])
                    elif k == 2:
                        stage = work.tile([128, 2 * CHUNK], F32, tag="stage")
                        stats_and_store(k, ch, npx, ps,
                                        lambda ot: stage[:, :npx])
                        nc.gpsimd.dma_start(y2d[:, n0:n0 + npx], stage[:, :npx])
                    else:
                        stats_and_store(k, ch, npx, ps,
                                        lambda ot: y3sb[:, n0:n0 + npx])
                finalize_stats(k, work)
                if trace_scopes:
                    nc.leave_named_scope(sc)

            # ===== phase 4: y3 -> upper-tri logits U =======================
            if trace_scopes:
                sc = nc.enter_named_scope("phase4")
            for ch in range(NCHUNK):
                n0, npx, segs = _chunk_geom(ch)
                z = work.tile([128, CHUNK], F32, tag="z4")
                nc.scalar.activation(z[:, :npx], y3sb[:, n0:n0 + npx], AF.Lrelu,
                                     bias=t_sb[3][:, 0:1], scale=s_sb[3][:, 0:1],
                                     alpha=SLOPE)
                lp = psum1.tile([1, CHUNK], F32, tag="lp")
                nc.tensor.matmul(lp[0:1, :npx], pksb[:, _W4:_W4 + 1], z[:, :npx],
                                 start=True, stop=True)
                lst = work.tile([1, CHUNK], F32, tag="lst")
                nc.vector.tensor_copy(lst[0:1, :npx], lp[0:1, :npx])
                for (row, col0, w, pos) in segs:
                    nc.gpsimd.dma_start(u_d[row:row + 1, col0:col0 + w],
                                        lst[0:1, pos:pos + w])
            if trace_scopes:
                nc.leave_named_scope(sc)

        # ===== phase 4.5 + 5: mirror logits, softmax + topk mask ===========
        if trace_scopes:
            sc = nc.enter_named_scope("phase5")
        NR = (K // 8) + 1  # 13 max8 rounds to reach rank 100
        with tc.tile_pool(name="smax", bufs=2) as smax:
            # load U (upper triangular, exact zeros below the diagonal)
            nv = V - 128  # 73
            ut = [smax.tile([128, V], F32, tag=f"ut{j}", name=f"ut{j}") for j in range(2)]
            nc.sync.dma_start(ut[0][:, :], u_d[0:128, :])
            nc.sync.dma_start(ut[1][:nv, :], u_d[128:V, :])
            # transpose U blockwise on the PE (reusing the conv psum ring),
            # strict-lower-mask, add: A = U + strict_lower(U^T)
            at = [smax.tile([128, V], F32, tag=f"at{j}", name=f"at{j}") for j in range(2)]
            ptA = psum.tile([128, CHUNK], F32, tag="ps", name="ps")
            ptB = psum.tile([128, CHUNK], F32, tag="ps", name="ps")
            ptC = psum.tile([128, CHUNK], F32, tag="ps", name="ps")
            ptD = psum.tile([128, CHUNK], F32, tag="ps", name="ps")
            nc.tensor.transpose(ptA[:, 0:128], ut[0][:, 0:128], ident)
            nc.tensor.transpose(ptB[:, 0:nv], ut[1][0:nv, 0:128],
                                pksb[0:nv, _IDC:_IDC + nv])
            nc.tensor.transpose(ptC[0:nv, 0:128], ut[0][:, 128:V], ident)
            nc.tensor.transpose(ptD[0:nv, 0:nv], ut[1][0:nv, 128:V],
                                pksb[0:nv, _IDC:_IDC + nv])
            utt0 = smax.tile([128, V], F32, tag="utt0")
            utt1 = smax.tile([128, V], F32, tag="utt1")
            nc.vector.tensor_copy(utt0[:, 0:128], ptA[:, 0:128])
            nc.vector.tensor_copy(utt0[:, 128:V], ptB[:, 0:nv])
            nc.vector.tensor_copy(utt1[0:nv, 0:128], ptC[0:nv, 0:128])
            nc.vector.tensor_copy(utt1[0:nv, 128:V], ptD[0:nv, 0:nv])
            nc.gpsimd.tensor_tensor(utt0[:, :], utt0[:, :], mask0, op=ALU.mult)
            nc.vector.tensor_tensor(at[0][:, :], ut[0][:, :], utt0[:, :], op=ALU.add)
            nc.gpsimd.tensor_tensor(utt1[:nv, :], utt1[:nv, :], mask1[:nv, :],
                                    op=ALU.mult)
            nc.vector.tensor_tensor(at[1][:nv, :], ut[1][:nv, :],
                                    utt1[:nv, :], op=ALU.add)

            for rt, (r0, nr) in enumerate([(0, 128), (128, V - 128)]):
                lt = at[rt]
                lc = smax.tile([128, V], F32, tag="lc")
                nc.vector.tensor_copy(lc[:nr, :], lt[:nr, :])
                mx = smax.tile([128, 8 * NR], F32, tag="mx")
                for r in range(NR):
                    nc.vector.max(mx[:nr, 8 * r: 8 * (r + 1)], lc[:nr, :])
                    if r < NR - 1:
                        nc.vector.match_replace(lc[:nr, :],
                                                mx[:nr, 8 * r: 8 * (r + 1)],
                                                lc[:nr, :], -1e30)
                nmx = smax.tile([128, 1], F32, tag="nmx")
                nc.vector.tensor_scalar_mul(nmx[:nr, :], mx[:nr, 0:1], -1.0)
                et = smax.tile([128, V], F32, tag="et")
                rsum = smax.tile([128, 1], F32, tag="rsum")
                nc.scalar.activation(et[:nr, :], lt[:nr, :], AF.Exp,
                                     bias=nmx[:nr, 0:1], scale=1.0,
                                     accum_out=rsum[:nr, 0:1])
                rec = smax.tile([128, 1], F32, tag="rec")
                nc.vector.reciprocal(rec[:nr, :], rsum[:nr, :])
                pt = smax.tile([128, V], F32, tag="pt")
                nc.vector.tensor_scalar_mul(pt[:nr, :], et[:nr, :], rec[:nr, 0:1])
                ot_ = smax.tile([128, V], F32, tag="ot")
                nc.vector.scalar_tensor_tensor(ot_[:nr, :], lt[:nr, :],
                                               mx[:nr, K - 1:K], pt[:nr, :],
                                               op0=ALU.is_ge, op1=ALU.mult)
                nc.sync.dma_start(out_d[r0:r0 + nr, :], ot_[:nr, :])
        if trace_scopes:
            nc.leave_named_scope(sc)

    nc.finalize()
    return nc


def _prep_inputs(inputs):
    """Host-side packing of the full inputs into one [128, NCOLS] per-core
    array: x^T (core's batch element) | conv weights | BN params."""
    x = np.ascontiguousarray(inputs["x"], dtype=np.float32)

    def ctile(w):  # [cout, cin] -> [128, cin/128 * cout] packed per cin-tile
        wT = np.ascontiguousarray(np.asarray(w, np.float32).T)  # [cin, cout]
        cin, cout = wT.shape
        return wT.reshape(cin // 128, 128, cout).transpose(1, 0, 2).reshape(128, -1)

    def pcols(v):  # [cout] -> [128, cout/128]
        return np.asarray(v, np.float32).reshape(-1, 128).T

    parts = [ctile(inputs[f"w{k}"]) for k in range(5)]
    for k in range(4):
        parts.append(np.concatenate(
            [pcols(inputs[f"g{k}"]), pcols(inputs[f"be{k}"])], axis=1))
    parts.append(np.eye(128, dtype=np.float32))
    p_idx = np.arange(128, dtype=np.float32)[:, None]
    c_idx = np.arange(V, dtype=np.float32)[None, :]
    parts.append((c_idx < p_idx).astype(np.float32))          # rows 0..127
    parts.append((c_idx < p_idx + 128).astype(np.float32))    # rows 128..200
    shared = np.concatenate(parts, axis=1)
    assert shared.shape == (128, NCOLS - 2 * V), shared.shape

    in_maps = []
    for c in range(N_CORES):
        xt = x[c].T.reshape(2, 128, V).transpose(1, 0, 2).reshape(128, 2 * V)
        in_maps.append({"pk": np.ascontiguousarray(
            np.concatenate([xt, shared], axis=1), np.float32)})
    return in_maps


class _Runner:
    """Cached PJRT executor for the bass module.

    Functionally the same axon path as bass_utils.run_bass_kernel_spmd
    (shard_map over 8 neuron devices + bass_exec custom call), but the
    jitted executable is built ONCE and reused — run_bass_kernel_spmd
    rebuilds the jax.jit closure per call, paying a full retrace/relower
    (~700ms) on every invocation. The donated-zero-output trick is also
    dropped: this kernel writes every element of its output, so the
    dummy output-shaped operands can be persistent device arrays instead
    of per-call zero uploads.
    """

    def __init__(self, nc):
        import jax
        from jax.sharding import Mesh, PartitionSpec, NamedSharding
        from jax.experimental.shard_map import shard_map
        from concourse.bass2jax import (
            _bass_exec_p,
            partition_id_tensor,
            install_neuronx_cc_hook,
        )

        install_neuronx_cc_hook()
        self.jax = jax
        self.nc = nc
        if nc.dbg_addr is not None and nc.dbg_callbacks:
            raise RuntimeError("dbg callbacks unsupported under axon")
        self.dbg_name = nc.dbg_addr.name if nc.dbg_addr is not None else None

        partition_name = (
            nc.partition_id_tensor.name if nc.partition_id_tensor else None
        )
        in_names, out_names, out_avals, zero_shapes = [], [], [], []
        for alloc in nc.m.functions[0].allocations:
            if not isinstance(alloc, mybir.MemoryLocationSet):
                continue
            name = alloc.memorylocations[0].name
            if alloc.kind == "ExternalInput":
                if name != partition_name:
                    in_names.append(name)
            elif alloc.kind == "ExternalOutput":
                out_names.append(name)
                shape = tuple(alloc.tensor_shape)
                dtype = mybir.dt.np(alloc.dtype)
                out_avals.append(jax.core.ShapedArray(shape, dtype))
                zero_shapes.append((shape, dtype))
        n_params = len(in_names)
        n_outs = len(out_avals)
        all_in_names = list(in_names) + list(out_names)
        if partition_name is not None:
            all_in_names.append(partition_name)
        self.in_names, self.out_names, self.out_avals = in_names, out_names, out_avals

        devices = jax.devices()[:N_CORES]
        assert len(devices) == N_CORES
        self.mesh = Mesh(np.asarray(devices), ("core",))
        self.sharding = NamedSharding(self.mesh, PartitionSpec("core"))

        def _body(*args):
            operands = list(args)
            if partition_name is not None:
                operands.append(partition_id_tensor())
            return tuple(
                _bass_exec_p.bind(
                    *operands,
                    out_avals=tuple(out_avals),
                    in_names=tuple(all_in_names),
                    out_names=tuple(out_names),
                    lowering_input_output_aliases=(),
                    sim_require_finite=True,
                    sim_require_nnan=True,
                    nc=nc,
                )
            )

        self._sharded = jax.jit(
            shard_map(
                _body,
                mesh=self.mesh,
                in_specs=(PartitionSpec("core"),) * (n_params + n_outs),
                out_specs=(PartitionSpec("core"),) * n_outs,
                check_rep=False,
            ),
            keep_unused=True,
        )
        # persistent dummy operands for the output slots (never read: the
        # kernel fully writes its outputs)
        self._zeros = [
            jax.device_put(
                np.zeros((N_CORES * s[0],) + tuple(s[1:]), dt), self.sharding
            )
            for (s, dt) in zero_shapes
        ]

    def concat_inputs(self, in_maps):
        if self.dbg_name is not None:
            dbg = np.zeros((1, 2), np.uint32)
            in_maps = [{**m, self.dbg_name: dbg} for m in in_maps]
        return [
            np.concatenate([np.asarray(m[nm]) for m in in_maps], axis=0)
            for nm in self.in_names
        ]

    def put_inputs(self, concat_in):
        return [self.jax.device_put(a, self.sharding) for a in concat_in]

    def dispatch(self, dev_in):
        """one kernel execution on the 8 cores (async; returns device arrays)"""
        return self._sharded(*dev_in, *self._zeros)

    def run_full(self, in_maps):
        outs = self.dispatch(self.put_inputs(self.concat_inputs(in_maps)))
        n = N_CORES
        return [
            {
                nm: np.asarray(outs[i]).reshape(n, *self.out_avals[i].shape)[c]
                for i, nm in enumerate(self.out_names)
            }
            for c in range(n)
        ]


_RUNNER = None


def _get_runner():
    global _RUNNER
    if _RUNNER is None:
        _RUNNER = _Runner(_build_nc())
    return _RUNNER


def kernel(**inputs):
    r = _get_runner()
    res = r.run_full(_prep_inputs(inputs))
    return np.stack([res[c]["outb"] for c in range(N_CORES)], axis=0)

